# revision 36
# baseline (speedup 1.0000x reference)
"""Trainium2 Bass kernel v6 for nn_BPR_76665166234050 (3-hop LightGCN + BPR).

Strategy (8 NeuronCores, SPMD):
- Destination-sharded spmm per hop; per-core exact segment sums (no psum AR).
- 4 gather passes instead of 6: rounds 2 and 3 fuse two hops each by
  interleaving the two source tables into 128-wide bf16 rows, so one
  256B-token gather feeds two PSUM accumulations.
    P1 (U): g1u = A @ i0              src [i0 | 0]      bf16 [IPAD,128]
    P2 (I): g1i = At @ u0, g2i = At @ g1u   src [u0 | g1u] bf16 [UPAD,128]
    P3 (U): g2u = A @ g1i, g3u = A @ g2i    src [g1i | g2i] bf16 [IPAD,128]
    P4 (I): g3i = At @ g2u            src [g2u | 0]     bf16 [UPAD,128]
- Gathers: nc.gpsimd.dma_gather of 256B bf16 rows, round-robined over 4
  SWDGE queues (ucode max) so the 4 queue rings drain concurrently. Pad
  token slots carry valid index 0 (zero one-hot column), so num_idxs_reg is
  the static cap — no count registers, no serialization.
- One-hot*val windows ([128,64] bf16 per (chunk,half)) are HOST-precomputed
  and DMA'd per block; PE does one LDW + 1-2 matmuls per window into a
  [128,64] f32 PSUM per block.
- Host serpentine degree-balancing permutation of user/item ids equalizes
  per-(core,blk,sch) token counts (outputs are permutation-invariant).
- AllGather (bf16) between passes, split 75/25 so only a small AG sits on
  each pass boundary; tail AG tables in bf16; U-side self-distillation
  overlaps P4. gcn accumulators stay f32 in SBUF.
HW: 4.82ms, rel err 1.2e-5 (baseline 14.87ms).
"""
import sys
sys.path.insert(0, "/opt/trn_rl_repo")
import numpy as np
import ml_dtypes

BF16 = ml_dtypes.bfloat16
# 4 SWDGE queues (ucode max) — round-robin gathers across queue rings so
# their drains overlap; single_packet=True crashes HW, keep False.
NQ = 4
SP = False


def _rup(x, m):
    return (x + m - 1) // m * m


class CFG:
    def __init__(self, user=100000, item=50000, d=64, e=3200000, b=16384,
                 ncores=8, chunk=25088):
        self.USER, self.ITEM, self.D, self.E, self.B = user, item, d, e, b
        self.NC = ncores
        self.CHUNK = chunk
        self.UPAD = _rup(user, 128 * ncores)
        self.IPAD = _rup(item, 128 * ncores)
        self.UCH = (self.UPAD + chunk - 1) // chunk
        self.ICH = (self.IPAD + chunk - 1) // chunk
        self.USH = self.UPAD // ncores
        self.ISH = self.IPAD // ncores
        self.UBLK = self.USH // 128
        self.IBLK = self.ISH // 128
        self.BSH = b // ncores
        assert b % (16 * ncores) == 0


def _pieces(nblk):
    if nblk < 2:
        return [(0, nblk)]
    h = max(1, min(nblk - 1, (nblk * 3) // 4))
    return [(0, h), (h, nblk)]


def _src_segmap(cfg, src, sh_rows, pieces):
    """Map source row ids to (seg, sloc) over the piece-ordered table.

    Piece-ordered table: rows = concat over pieces of concat over cores of
    the core's piece-slice. Returns seg, sloc, specs [(piece, lo, sz)],
    bases [row base of each piece in a single-tensor layout].
    """
    core_s = src // sh_rows
    m = src % sh_rows
    seg = np.zeros(len(src), np.int64)
    sloc = np.zeros(len(src), np.int64)
    specs, bases = [], []
    seg_base = 0
    tbl_base = 0
    for pidx, (b0, b1) in enumerate(pieces):
        rlo, rhi = b0 * 128, b1 * 128
        prows = (rhi - rlo) * cfg.NC
        nch = (prows + 32767) // 32768 if prows > 32767 else 1
        nch = max(nch, (prows + 32767) // 32767)
        csz = _rup((prows + nch - 1) // nch, 16)
        msk = (m >= rlo) & (m < rhi)
        pos = core_s[msk] * (rhi - rlo) + (m[msk] - rlo)
        seg[msk] = seg_base + pos // csz
        sloc[msk] = pos % csz
        for c in range(nch):
            lo = c * csz
            specs.append((pidx, lo, min(csz, prows - lo)))
        bases.append(tbl_base)
        seg_base += nch
        tbl_base += prows
    return seg, sloc, specs, bases


def _prep_direction(cfg, dst, seg_in, sloc_in, val, sh_rows, nblk, nsch):
    """Token schedule for one direction (dst-sharded).

    Tokens sorted by (core, blk, sch, dloc); slot (p=rank%128,
    chunk=rank//128) within each (blk,sch) group, caps uniform across cores.
    Windows: per chunk, the half-blocks (0-63 / 64-127) its tokens touch in
    ANY core; one-hot = is_eq(dloc-64h, iota64)*val masks naturally.

    Returns (meta, per_core):
      meta: blocks -> dict(goffs=[(choff,cap)]*nsch, wins=[(cj,half)],
            gstart/gstop flags, woff, nch)
      per_core: idx16 [128,T/16] i16, wdst/wval [128,NW] bf16
    """
    NC = cfg.NC
    core = dst // sh_rows
    blk = (dst % sh_rows) // 128
    dloc = dst % 128
    sch = seg_in
    sloc = sloc_in.astype(np.int16)

    order = np.lexsort((dloc, sch, blk, core))
    core, blk, dloc, sch, sloc, val = (a[order] for a in
                                       (core, blk, dloc, sch, sloc, val))

    key = (core * nblk + blk) * nsch + sch
    ncell = NC * nblk * nsch
    counts = np.bincount(key, minlength=ncell).reshape(NC, nblk, nsch)
    caps = np.maximum(_rup(counts.max(axis=0), 128), 128)   # [nblk, nsch]

    # chunk offsets (in tokens) per (blk, sch)
    offs = np.zeros((nblk, nsch), np.int64)
    t = 0
    for b in range(nblk):
        for c in range(nsch):
            offs[b, c] = t
            t += caps[b, c]
    T = t
    NCHT = T // 128

    seg_start = np.zeros(ncell + 1, np.int64)
    np.cumsum(np.bincount(key, minlength=ncell), out=seg_start[1:])
    rank = np.arange(len(dst)) - seg_start[key]
    pos = offs[blk, sch] + rank
    cid = pos // 128          # global chunk id within core slab
    p = pos % 128

    idx_all = np.zeros((NC, T), np.int16)
    val_all = np.zeros((NC, T), np.float32)
    dloc_all = np.full((NC, T), -1, np.int16)
    idx_all[core, pos] = sloc
    val_all[core, pos] = val
    dloc_all[core, pos] = dloc

    # per-chunk half membership unioned over cores
    lo = np.full(NCHT, 127, np.int64)
    hi = np.zeros(NCHT, np.int64)
    np.minimum.at(lo, cid, dloc)
    np.maximum.at(hi, cid, dloc)
    has0 = lo < 64
    has1 = hi >= 64

    blocks = []
    nw = 0
    for b in range(nblk):
        c0 = offs[b, 0] // 128
        cend = (offs[b, 0] + caps[b].sum()) // 128
        wins, gstart, gstop = [], [], []
        for h in range(2):
            hs = [cj - c0 for cj in range(c0, cend)
                  if (has0[cj] if h == 0 else has1[cj])]
            if not hs:
                hs = [0]          # dummy: all-masked window still clears psum
            for i, cj in enumerate(hs):
                wins.append((cj, h))
                gstart.append(i == 0)
                gstop.append(i == len(hs) - 1)
        mins = counts.min(axis=0)
        blocks.append(dict(
            c0=c0, nch=cend - c0, woff=nw, wins=wins,
            gstart=gstart, gstop=gstop,
            goffs=[(int(offs[b, c]), int(caps[b, c])) for c in range(nsch)],
            glos=[int(mins[b, c]) // 128 for c in range(nsch)],
        ))
        nw += len(wins)

    # window descriptors (same for all cores): absolute chunk + half
    cj_abs = np.array([bm["c0"] + cj for bm in blocks
                       for (cj, h) in bm["wins"]], np.int64)
    hh = np.array([h for bm in blocks for (cj, h) in bm["wins"]], np.int64)

    per_core = []
    for k in range(NC):
        dw = dloc_all[k].reshape(NCHT, 128).T  # [128, NCHT] int16
        vw = val_all[k].reshape(NCHT, 128).T   # [128, NCHT] f32
        d = dw[:, cj_abs].astype(np.int64) - 64 * hh[None, :]  # [128, nw]
        v = vw[:, cj_abs]
        m = (d >= 0) & (d < 64)
        wone = np.zeros((128, nw, 64), BF16)
        pp, ww = np.nonzero(m)
        wone[pp, ww, d[pp, ww]] = v[pp, ww].astype(BF16)
        iw = idx_all[k].reshape(T // 16, 16).T
        idx16 = np.tile(iw, (8, 1))
        per_core.append(dict(idx=idx16, wone=wone.reshape(128, nw * 64)))

    meta = dict(T=T, NW=nw, blocks=blocks, nsch=nsch, nblk=nblk)
    return meta, per_core


def _wrap_shard(tbl_pad, k, sh_rows):
    s = tbl_pad[k * sh_rows:(k + 1) * sh_rows]
    nb = sh_rows // 128
    d = s.shape[1]
    return s.reshape(nb, 128, d).transpose(1, 0, 2).reshape(128, nb * d).copy()


def _wrap_vec(vec_pad, k, sh_rows):
    s = vec_pad[k * sh_rows:(k + 1) * sh_rows]
    nb = sh_rows // 128
    return s.reshape(nb, 128).T.copy()


def _wrap_idx(ix, n):
    w = ix.astype(np.int16).reshape(n // 16, 16).T
    return np.tile(w, (8, 1))


def build_program(cfg, mu, mi, with_tail=True):
    import concourse.bass as bass
    import concourse.bacc as bacc
    import concourse.tile as tile
    from concourse import mybir

    D, NC = cfg.D, cfg.NC
    f32, bf16, i16 = mybir.dt.float32, mybir.dt.bfloat16, mybir.dt.int16
    AOT = mybir.AluOpType

    nc = bacc.Bacc("TRN2", target_bir_lowering=False, debug=False,
                   num_devices=NC, num_swdge_queues=NQ,
                   dynamic_dma_scratch_size=24576)

    # ---- I/O ----
    i0p = nc.dram_tensor("i0p", [cfg.IPAD, 128], bf16, kind="ExternalInput")
    u0h = nc.dram_tensor("u0h", [cfg.USH, 128], bf16,
                         kind="ExternalInput")   # u0 half prefilled
    uemb_sh = nc.dram_tensor("uemb_sh", [128, cfg.UBLK * D], f32, kind="ExternalInput")
    iemb_sh = nc.dram_tensor("iemb_sh", [128, cfg.IBLK * D], f32, kind="ExternalInput")
    oldu_sh = nc.dram_tensor("oldu_sh", [128, cfg.UBLK * D], f32, kind="ExternalInput")
    oldi_sh = nc.dram_tensor("oldi_sh", [128, cfg.IBLK * D], f32, kind="ExternalInput")
    nu_sh = nc.dram_tensor("nu_sh", [128, cfg.UBLK], f32, kind="ExternalInput")
    ni_sh = nc.dram_tensor("ni_sh", [128, cfg.IBLK], f32, kind="ExternalInput")
    idx_u = nc.dram_tensor("idx_u", [128, mu["T"] // 16], i16, kind="ExternalInput")
    idx_i = nc.dram_tensor("idx_i", [128, mi["T"] // 16], i16, kind="ExternalInput")
    wone_u = nc.dram_tensor("wone_u", [128, mu["NW"] * 64], bf16,
                            kind="ExternalInput")
    wone_i = nc.dram_tensor("wone_i", [128, mi["NW"] * 64], bf16,
                            kind="ExternalInput")
    BSH = cfg.BSH
    bidx_u = nc.dram_tensor("bidx_u", [128, BSH // 16], i16, kind="ExternalInput")
    bidx_i = nc.dram_tensor("bidx_i", [128, BSH // 16], i16, kind="ExternalInput")
    bidx_j = nc.dram_tensor("bidx_j", [128, BSH // 16], i16, kind="ExternalInput")
    bmask = nc.dram_tensor("bmask", [128, (BSH // 128) * 8], bf16, kind="ExternalInput")
    ones_in = nc.dram_tensor("ones", [128, 1], f32, kind="ExternalInput")
    out_d = nc.dram_tensor("out", [4], f32, kind="ExternalOutput")

    # ---- internal DRAM ----
    def internal(name, rows, cols, dt=bf16, shared=False):
        kw = {"addr_space": "Shared"} if shared else {}
        return nc.dram_tensor(name, [rows, cols], dt, kind="Internal", **kw)

    pcs_u, pcs_i = mu["dst_pieces"], mi["dst_pieces"]

    def piece_pair(name, pieces):
        agins, fulls = [], []
        for pidx, (b0, b1) in enumerate(pieces):
            sh = (b1 - b0) * 128
            agins.append(internal(f"agin_{name}{pidx}", sh, 128))
            fulls.append(internal(f"{name}_full{pidx}", sh * NC, 128,
                                  shared=True))
        return agins, fulls

    agin_ug1, ug1_full = piece_pair("ug1", pcs_u)
    agin_gg, gg_full = piece_pair("gg", pcs_i)
    agin_g2u, g2u_full = piece_pair("g2u", pcs_u)
    agin_gcu = internal("agin_gcu", cfg.USH, D)
    gcu_full = internal("gcu_full", cfg.UPAD, D, shared=True)
    agin_gci = internal("agin_gci", cfg.ISH, D)
    gci_full = internal("gci_full", cfg.IPAD, D, shared=True)

    maxch = max(max(b["nch"] for b in mu["blocks"]),
                max(b["nch"] for b in mi["blocks"]))
    maxw = max(max(len(b["wins"]) for b in mu["blocks"]),
               max(len(b["wins"]) for b in mi["blocks"]))

    with tile.TileContext(nc) as tc:
        with (
            tc.tile_pool(name="persist", bufs=1) as pp,
            tc.tile_pool(name="io", bufs=3) as iop,
            tc.tile_pool(name="gath", bufs=3) as gp,
            tc.tile_pool(name="lhs", bufs=3) as lp,
            tc.tile_pool(name="drain", bufs=4) as dp,
            tc.tile_pool(name="psum", bufs=3, space="PSUM") as psp,
            tc.tile_pool(name="psumB", bufs=3, space="PSUM") as pspB,
            tc.tile_pool(name="psum4", bufs=1, space="PSUM") as psp4,
            tc.tile_pool(name="tail", bufs=1) as tp,
        ):
            gcn_u = pp.tile([128, cfg.UBLK, D], f32, tag="gcn_u")
            gcn_i = pp.tile([128, cfg.IBLK, D], f32, tag="gcn_i")
            qctr = [0]

            for pidx, (b0, b1) in enumerate(pcs_u):
                nc.sync.dma_start(agin_ug1[pidx].ap(),
                                  u0h.ap()[b0 * 128:b1 * 128, :])
            nc.sync.dma_start(gcn_u[:], uemb_sh.ap().rearrange(
                "p (b d) -> p b d", d=D))
            nc.sync.dma_start(gcn_i[:], iemb_sh.ap().rearrange(
                "p (b d) -> p b d", d=D))

            def gat(g_t, srcs, idx_t, bm, b, nsch):
                """Emit gathers for one block: per src segment, split into
                sub-gathers that fit one SWDGE queue ring (1024 descs)."""
                RING = 1536
                off0 = bm["goffs"][0][0]
                for c, (off, cap) in enumerate(bm["goffs"]):
                    if cap == 0:
                        continue
                    rel = off - off0
                    s_t, s_lo, s_hi = srcs[c]
                    # halve only gathers that overflow one queue ring
                    sub = cap if cap <= RING else _rup((cap + 1) // 2, 128)
                    for o in range(0, cap, sub):
                        sc = min(sub, cap - o)
                        ro = rel + o
                        q = qctr[0] % NQ
                        qctr[0] += 1
                        nc.gpsimd.dma_gather(
                            g_t[:, ro // 128:(ro + sc) // 128, :],
                            s_t.ap()[s_lo:s_hi, :],
                            idx_t[:, ro // 16:(ro + sc) // 16],
                            num_idxs=sc,
                            num_idxs_reg=sc,
                            elem_size=128,
                            single_packet=SP,
                            queue_num=q,
                        )

            def hop_pass(meta, idx_d, wone_d, src_tensors, fused,
                         wgtA, accA, wgtB=None, accB=None,
                         tblA=None, tblA_col=None, tblB=None, tblB_col=None,
                         ag_after=None):
                """One gather pass. fused: two hops from row halves.

                src_tensors: one tensor per source piece (or a single tensor
                covering all pieces); tblA/tblB: list of per-piece agin
                tensors aligned with this direction's dst pieces; ag_after:
                {last_block_of_piece: [(agin, agout)]} collectives to emit.
                """
                specs = meta["specs"]
                if not isinstance(src_tensors, (list, tuple)):
                    bases = []
                    base = 0
                    for pidx, (b0, b1) in enumerate(meta["src_pieces"]):
                        bases.append(base)
                        base += (b1 - b0) * 128 * NC
                    srcs = [(src_tensors, bases[p] + lo, bases[p] + lo + sz)
                            for (p, lo, sz) in specs]
                else:
                    srcs = [(src_tensors[p], lo, lo + sz)
                            for (p, lo, sz) in specs]
                dpieces = meta["dst_pieces"]

                def tbl_at(tbl_list, b):
                    for (b0, b1), t in zip(dpieces, tbl_list):
                        if b0 <= b < b1:
                            return t, (b - b0) * 128
                    raise AssertionError

                for b, bm in enumerate(meta["blocks"]):
                    Tb = sum(c for _, c in bm["goffs"])
                    nwb = len(bm["wins"])
                    off0 = bm["goffs"][0][0]
                    idx_t = iop.tile([128, (maxch * 128) // 16], i16, tag="idx")
                    nc.sync.dma_start(
                        idx_t[:, :Tb // 16],
                        idx_d.ap()[:, off0 // 16:(off0 + Tb) // 16])

                    g_t = gp.tile([128, maxch, 128], bf16, tag="g")
                    gat(g_t, srcs, idx_t, bm, b, meta["nsch"])

                    l_t = lp.tile([128, maxw, 64], bf16, tag="l")
                    nc.sync.dma_start(
                        l_t[:, :nwb, :],
                        wone_d.ap()[:, bm["woff"] * 64:(bm["woff"] + nwb) * 64]
                        .rearrange("p (w j) -> p w j", j=64))

                    psA = psp.tile([128, D], f32, tag="psA")
                    if fused:
                        psB = pspB.tile([128, D], f32, tag="psB")
                    else:
                        psB = None
                    for w, (cj, h) in enumerate(bm["wins"]):
                        st, sp = bm["gstart"][w], bm["gstop"][w]
                        nc.tensor.matmul(
                            psA[64 * h:64 * h + 64, :],
                            l_t[:, w, :],
                            g_t[:, cj, 0:64],
                            start=st, stop=sp,
                            tile_position=(0, 64 * h),
                        )
                        if fused:
                            nc.tensor.matmul(
                                psB[64 * h:64 * h + 64, :],
                                l_t[:, w, :],
                                g_t[:, cj, 64:128],
                                start=st, stop=sp,
                                tile_position=(0, 64 * h),
                            )

                    def drain(ps, wgt, acc, tbl, tbl_col):
                        nc.vector.scalar_tensor_tensor(
                            acc[:, b, :], ps[:], float(wgt),
                            acc[:, b, :], AOT.mult, AOT.add)
                        if tbl is not None:
                            t, r0 = tbl_at(tbl, b)
                            cv = dp.tile([128, D], bf16, tag="cv")
                            nc.scalar.copy(cv[:], ps[:])
                            nc.sync.dma_start(
                                t.ap()[r0:r0 + 128, tbl_col:tbl_col + D],
                                cv[:])
                    drain(psA, wgtA, accA, tblA, tblA_col)
                    if fused:
                        drain(psB, wgtB, accB, tblB, tblB_col)
                    if ag_after and b in ag_after:
                        for gi, go in ag_after[b]:
                            allgather(gi, go)

            def allgather(ag_in, ag_out):
                nc.gpsimd.collective_compute(
                    "AllGather", mybir.AluOpType.bypass,
                    replica_groups=[list(range(NC))],
                    ins=[ag_in.ap()], outs=[ag_out.ap()],
                )

            U = (mu, idx_u, wone_u)
            I = (mi, idx_i, wone_i)

            def ag_map(pieces, agins, fulls):
                return {b1 - 1: [(agins[p], fulls[p])]
                        for p, (b0, b1) in enumerate(pieces)}

            def self_loss(acc_tile, old_d, n_d, nblk, col):
                CH = min(7, nblk)
                nv_t = tp.tile([128, nblk], f32, tag=f"nv{col}")
                rs = tp.tile([128, nblk], f32, tag=f"rs{col}")
                nc.sync.dma_start(nv_t[:], n_d.ap())
                for o in range(0, nblk, CH):
                    ch = min(CH, nblk - o)
                    old_t = iop.tile([128, CH, D], f32, tag="oldc")
                    nc.sync.dma_start(
                        old_t[:, :ch, :],
                        old_d.ap()[:, o * D:(o + ch) * D]
                        .rearrange("p (b d) -> p b d", d=D))
                    nc.vector.tensor_tensor(old_t[:, :ch, :],
                                            acc_tile[:, o:o + ch, :],
                                            old_t[:, :ch, :], AOT.subtract)
                    nc.vector.tensor_tensor(old_t[:, :ch, :],
                                            old_t[:, :ch, :],
                                            old_t[:, :ch, :], AOT.mult)
                    nc.vector.tensor_reduce(rs[:, o:o + ch], old_t[:, :ch, :],
                                            mybir.AxisListType.X, AOT.add)
                nc.scalar.activation(rs[:], rs[:],
                                     mybir.ActivationFunctionType.Sqrt)
                nc.vector.tensor_tensor(rs[:], rs[:], nv_t[:], AOT.mult)
                nc.vector.tensor_reduce(part_t[:, col:col + 1], rs[:],
                                        mybir.AxisListType.X, AOT.add)

            self_loss_fns = [
                lambda: self_loss(gcn_u, oldu_sh, nu_sh, cfg.UBLK, 2),
                lambda: self_loss(gcn_i, oldi_sh, ni_sh, cfg.IBLK, 3),
            ]

            # P1: g1u = A @ i0
            hop_pass(*U, i0p, False, 0.5, gcn_u,
                     tblA=agin_ug1, tblA_col=64,
                     ag_after=ag_map(pcs_u, agin_ug1, ug1_full))
            # P2: g1i = At@u0 (cols 0:64), g2i = At@g1u (cols 64:128)
            hop_pass(*I, ug1_full, True, 0.5, gcn_i, 1.0 / 3.0,
                     gcn_i, tblA=agin_gg, tblA_col=0,
                     tblB=agin_gg, tblB_col=64,
                     ag_after=ag_map(pcs_i, agin_gg, gg_full))
            # P3: g2u = A@g1i, g3u = A@g2i (g3u drain fills unused col half)
            hop_pass(*U, gg_full, True, 1.0 / 3.0, gcn_u, 0.25,
                     gcn_u, tblA=agin_g2u, tblA_col=0,
                     tblB=agin_g2u, tblB_col=64,
                     ag_after=ag_map(pcs_u, agin_g2u, g2u_full))
            # gcn_u complete -> AG for tail (overlaps P4)
            gcu_b16 = tp.tile([128, cfg.UBLK, D], bf16, tag="gcu16")
            nc.scalar.copy(gcu_b16[:], gcn_u[:])
            nc.sync.dma_start(
                agin_gcu.ap().rearrange("(b p) d -> p b d", p=128), gcu_b16[:])
            allgather(agin_gcu, gcu_full)
            part_t = tp.tile([128, 4], f32, tag="part")
            if not with_tail:
                nc.vector.memset(part_t[:], 0.0)
            if with_tail:
                # U-side self-distillation overlaps P4 (gcn_u is final)
                self_loss_fns[0]()
            # P4: g3i = At @ g2u
            hop_pass(*I, g2u_full, False, 0.25, gcn_i)
            gci_b16 = tp.tile([128, cfg.IBLK, D], bf16, tag="gci16")
            nc.scalar.copy(gci_b16[:], gcn_i[:])
            nc.sync.dma_start(
                agin_gci.ap().rearrange("(b p) d -> p b d", p=128), gci_b16[:])
            allgather(agin_gci, gci_full)

            # ---------------- tail ----------------

            if with_tail:
                self_loss_fns[1]()

                BS = BSH // 128
                mask_t = tp.tile([128, 8 * BS], bf16, tag="bmask")
                nc.sync.dma_start(mask_t[:], bmask.ap())

                def batch_rows(src_full, rows_full, group, bidx_d, mask_lo,
                               ngrp, tag):
                    gt_full = tp.tile([128, BS * 4 * D], bf16, tag="bgshare")
                    gt = gt_full[:, :BS * group * D].rearrange(
                        "p (s gd) -> p s gd", gd=group * D)
                    bix_t = tp.tile([128, BSH // 16], i16, tag=f"bx{tag}")
                    nc.sync.dma_start(bix_t[:], bidx_d.ap())
                    src2 = src_full.ap().rearrange("(a g) d -> a (g d)",
                                                   g=group)
                    nc.gpsimd.dma_gather(
                        gt[:], src2, bix_t[:],
                        num_idxs=BSH, num_idxs_reg=BSH, elem_size=group * D,
                        single_packet=False)
                    rt = tp.tile([128, BS, D], f32, tag=f"br{tag}")
                    tmp = tp.tile([128, BS, D], f32, tag="btshare")
                    for q in range(ngrp):
                        m_b = mask_t[:, (mask_lo + q) * BS:
                                     (mask_lo + q + 1) * BS]\
                            .broadcast_to([128, BS, D])
                        dstt = rt if q == 0 else tmp
                        nc.vector.tensor_tensor(
                            dstt[:], gt[:, :, q * D:(q + 1) * D], m_b,
                            AOT.mult)
                        if q > 0:
                            nc.vector.tensor_tensor(rt[:], rt[:], tmp[:],
                                                    AOT.add)
                    return rt

                u_t = batch_rows(gcu_full, cfg.UPAD, 4, bidx_u, 0, 4, "u")
                ii_t = batch_rows(gci_full, cfg.IPAD, 2, bidx_i, 4, 2, "i")
                ij_t = batch_rows(gci_full, cfg.IPAD, 2, bidx_j, 6, 2, "j")

                pr = tp.tile([128, BS, D], f32, tag="pr")
                pi = tp.tile([128, BS], f32, tag="pi")
                pj = tp.tile([128, BS], f32, tag="pj")
                nc.vector.tensor_tensor(pr[:], u_t[:], ii_t[:], AOT.mult)
                nc.vector.tensor_reduce(pi[:], pr[:], mybir.AxisListType.X,
                                        AOT.add)
                nc.vector.tensor_tensor(pr[:], u_t[:], ij_t[:], AOT.mult)
                nc.vector.tensor_reduce(pj[:], pr[:], mybir.AxisListType.X,
                                        AOT.add)
                nc.vector.tensor_tensor(pi[:], pi[:], pj[:], AOT.subtract)
                bt = tp.tile([128, BS], f32, tag="bt2")
                nc.scalar.activation(bt[:], pi[:],
                                     mybir.ActivationFunctionType.Sigmoid)
                nc.scalar.activation(bt[:], bt[:],
                                     mybir.ActivationFunctionType.Ln,
                                     accum_out=part_t[:, 0:1])

                rg = tp.tile([128, BS], f32, tag="rg")
                rgt = tp.tile([128, BS], f32, tag="rgt")
                nc.vector.tensor_tensor(pr[:], u_t[:], u_t[:], AOT.mult)
                nc.vector.tensor_reduce(rg[:], pr[:], mybir.AxisListType.X,
                                        AOT.add)
                nc.vector.tensor_tensor(pr[:], ii_t[:], ii_t[:], AOT.mult)
                nc.vector.tensor_reduce(rgt[:], pr[:], mybir.AxisListType.X,
                                        AOT.add)
                nc.vector.tensor_tensor(rg[:], rg[:], rgt[:], AOT.add)
                nc.vector.tensor_tensor(pr[:], ij_t[:], ij_t[:], AOT.mult)
                nc.vector.tensor_reduce(rgt[:], pr[:], mybir.AxisListType.X,
                                        AOT.add)
                nc.vector.tensor_tensor(rg[:], rg[:], rgt[:], AOT.add)
                nc.vector.tensor_reduce(part_t[:, 1:2], rg[:],
                                        mybir.AxisListType.X, AOT.add)

            ones_t = tp.tile([128, 1], f32, tag="ones")
            nc.sync.dma_start(ones_t[:], ones_in.ap())
            ps4 = psp4.tile([4, 1], f32, tag="ps4")
            nc.tensor.matmul(ps4[:], part_t[:], ones_t[:],
                             start=True, stop=True)
            out_t = tp.tile([4, 1], f32, tag="out4")
            nc.scalar.copy(out_t[:], ps4[:])
            nc.sync.dma_start(out_d.ap().rearrange("(a b) -> a b", b=1),
                              out_t[:])

    nc.compile()
    return nc


def _balance_perm(deg, n, nc_, sh):
    """Serpentine-assign ids (ranked by degree) to (core, slot) so per-core
    per-degree-band token counts are nearly equal. Returns newpos[orig_id]
    -> position in the padded block-cyclic layout."""
    order = np.argsort(-deg, kind="stable")
    i = np.arange(n)
    grp = i // nc_
    lane = i % nc_
    core = np.where((grp % 2).astype(bool), nc_ - 1 - lane, lane)
    newpos = np.empty(n, np.int64)
    newpos[order] = core * sh + grp
    return newpos


def _preprocess(cfg, inputs):
    user = np.asarray(inputs["user"]).astype(np.int64)
    item_i = np.asarray(inputs["item_i"]).astype(np.int64)
    item_j = np.asarray(inputs["item_j"]).astype(np.int64)
    edge_u = np.asarray(inputs["edge_u"]).astype(np.int64)
    edge_i = np.asarray(inputs["edge_i"]).astype(np.int64)
    edge_val = np.asarray(inputs["edge_val"]).astype(np.float32)
    user_emb = np.asarray(inputs["user_emb"]).astype(np.float32)
    item_emb = np.asarray(inputs["item_emb"]).astype(np.float32)
    old_U = np.asarray(inputs["old_U_emb"]).astype(np.float32)
    old_I = np.asarray(inputs["old_I_emb"]).astype(np.float32)
    n_U = np.asarray(inputs["n_U"]).astype(np.float32)
    n_I = np.asarray(inputs["n_I"]).astype(np.float32)

    D = cfg.D

    posu = _balance_perm(np.bincount(edge_u, minlength=cfg.USER),
                         cfg.USER, cfg.NC, cfg.USH)
    posi = _balance_perm(np.bincount(edge_i, minlength=cfg.ITEM),
                         cfg.ITEM, cfg.NC, cfg.ISH)
    edge_u = posu[edge_u]
    edge_i = posi[edge_i]
    user = posu[user]
    item_i = posi[item_i]
    item_j = posi[item_j]

    def perm_rows(a, n, pos):
        out = np.zeros((n,) + a.shape[1:], a.dtype)
        out[pos] = a
        return out

    uemb_p = perm_rows(user_emb, cfg.UPAD, posu)
    iemb_p = perm_rows(item_emb, cfg.IPAD, posi)
    oldu_p = perm_rows(old_U, cfg.UPAD, posu)
    oldi_p = perm_rows(old_I, cfg.IPAD, posi)
    nu_p = perm_rows(n_U, cfg.UPAD, posu)
    ni_p = perm_rows(n_I, cfg.IPAD, posi)

    # piece layouts of the two source tables (by dst-shard block halves)
    pcs_u = _pieces(cfg.UBLK)
    pcs_i = _pieces(cfg.IBLK)
    # U direction: dst=user, src=item (item-table pieces)
    seg_u, sloc_u, specs_i, bases_i = _src_segmap(cfg, edge_i, cfg.ISH, pcs_i)
    mu, pc_u = _prep_direction(cfg, edge_u, seg_u, sloc_u, edge_val,
                               cfg.USH, cfg.UBLK, len(specs_i))
    # I direction: dst=item, src=user (user-table pieces)
    seg_i, sloc_i, specs_u, bases_u = _src_segmap(cfg, edge_u, cfg.USH, pcs_u)
    mi, pc_i = _prep_direction(cfg, edge_i, seg_i, sloc_i, edge_val,
                               cfg.ISH, cfg.IBLK, len(specs_u))
    mu["specs"], mu["src_pieces"], mu["dst_pieces"] = specs_i, pcs_i, pcs_u
    mi["specs"], mi["src_pieces"], mi["dst_pieces"] = specs_u, pcs_u, pcs_i

    # i0 in item-piece order
    i0p = np.zeros((cfg.IPAD, 128), BF16)
    base = 0
    for (b0, b1) in pcs_i:
        rlo, rhi = b0 * 128, b1 * 128
        psz = rhi - rlo
        for k in range(cfg.NC):
            i0p[base + k * psz:base + (k + 1) * psz, :D] = \
                iemb_p[k * cfg.ISH + rlo:k * cfg.ISH + rhi].astype(BF16)
        base += cfg.NC * psz

    ones = np.ones((128, 1), np.float32)

    in_maps = []
    BSH, BS = cfg.BSH, cfg.BSH // 128
    for k in range(cfg.NC):
        u0h = np.zeros((cfg.USH, 128), BF16)
        u0h[:, :D] = uemb_p[k * cfg.USH:(k + 1) * cfg.USH].astype(BF16)
        bs = slice(k * BSH, (k + 1) * BSH)
        bu, bi, bj = user[bs], item_i[bs], item_j[bs]
        masks = np.zeros((128, 8 * BS), BF16)
        for q in range(4):
            m = (bu % 4 == q).astype(np.float32).reshape(BS, 128).T
            masks[:, q * BS:(q + 1) * BS] = m
        for q in range(2):
            m = (bi % 2 == q).astype(np.float32).reshape(BS, 128).T
            masks[:, (4 + q) * BS:(5 + q) * BS] = m
            m = (bj % 2 == q).astype(np.float32).reshape(BS, 128).T
            masks[:, (6 + q) * BS:(7 + q) * BS] = m
        in_maps.append({
            "i0p": i0p, "u0h": u0h,
            "uemb_sh": _wrap_shard(uemb_p, k, cfg.USH),
            "iemb_sh": _wrap_shard(iemb_p, k, cfg.ISH),
            "oldu_sh": _wrap_shard(oldu_p, k, cfg.USH),
            "oldi_sh": _wrap_shard(oldi_p, k, cfg.ISH),
            "nu_sh": _wrap_vec(nu_p, k, cfg.USH),
            "ni_sh": _wrap_vec(ni_p, k, cfg.ISH),
            "idx_u": pc_u[k]["idx"], "wone_u": pc_u[k]["wone"],
            "idx_i": pc_i[k]["idx"], "wone_i": pc_i[k]["wone"],
            "ones": ones,
            "bidx_u": _wrap_idx(bu // 4, BSH),
            "bidx_i": _wrap_idx(bi // 2, BSH),
            "bidx_j": _wrap_idx(bj // 2, BSH),
            "bmask": masks,
        })
    return mu, mi, in_maps


def run(cfg, inputs, trace=False, use_sim=False, **bkw):
    from concourse import bass_utils
    mu, mi, in_maps = _preprocess(cfg, inputs)
    nc = build_program(cfg, mu, mi, **bkw)
    if use_sim:
        from concourse.bass_interp import MultiCoreSim
        sim = MultiCoreSim(nc, num_cores=cfg.NC, trace=False)
        cores = [sim.cores[i] for i in sorted(sim.cores)]
        for k, core in enumerate(cores):
            for name, arr in in_maps[k].items():
                core.tensor(name)[:] = arr
        sim.simulate(check_with_hw=False)

        class R:
            results = [{"out": np.array(core.tensor("out"))}
                       for core in cores]
            exec_time_ns = None
        res = R()
    else:
        res = bass_utils.run_bass_kernel_spmd(
            nc, in_maps, core_ids=list(range(cfg.NC)), trace=trace)
    parts = np.stack([res.results[k]["out"] for k in range(cfg.NC)])
    tot = parts.sum(axis=0)
    loss_bpr = -tot[0] / cfg.B + 1e-4 * tot[1] / cfg.B
    loss_self = tot[2] / cfg.USER + tot[3] / cfg.ITEM
    out = np.array([loss_bpr, 100.0 * loss_self, 1.0, 1.0], np.float32)
    return out, res


def kernel(**inputs):
    cfg = CFG()
    out, _ = run(cfg, inputs)
    return out



# revision 40
# speedup vs baseline: 1.0332x; 1.0332x over previous
"""Trainium2 Bass kernel v6 for nn_BPR_76665166234050 (3-hop LightGCN + BPR).

Strategy (8 NeuronCores, SPMD):
- Destination-sharded spmm per hop; per-core exact segment sums (no psum AR).
- 4 gather passes instead of 6: rounds 2 and 3 fuse two hops each by
  interleaving the two source tables into 128-wide bf16 rows, so one
  256B-token gather feeds two PSUM accumulations.
    P1 (U): g1u = A @ i0              src [i0 | 0]      bf16 [IPAD,128]
    P2 (I): g1i = At @ u0, g2i = At @ g1u   src [u0 | g1u] bf16 [UPAD,128]
    P3 (U): g2u = A @ g1i, g3u = A @ g2i    src [g1i | g2i] bf16 [IPAD,128]
    P4 (I): g3i = At @ g2u            src [g2u | 0]     bf16 [UPAD,128]
- Gathers: nc.gpsimd.dma_gather of 256B bf16 rows, round-robined over 4
  SWDGE queues (ucode max) so the 4 queue rings drain concurrently. Pad
  token slots carry valid index 0 (zero one-hot column), so num_idxs_reg is
  the static cap — no count registers, no serialization.
- One-hot*val windows ([128,64] bf16 per (chunk,half)) are HOST-precomputed
  and DMA'd per block; PE does one LDW + 1-2 matmuls per window into a
  [128,64] f32 PSUM per block.
- Host serpentine degree-balancing permutation of user/item ids equalizes
  per-(core,blk,sch) token counts (outputs are permutation-invariant).
- AllGather (bf16) between passes, split 75/25 so only a small AG sits on
  each pass boundary; tail AG tables in bf16; U-side self-distillation
  overlaps P4. gcn accumulators stay f32 in SBUF.
HW: 4.82ms, rel err 1.2e-5 (baseline 14.87ms).
"""
import sys
sys.path.insert(0, "/opt/trn_rl_repo")
import numpy as np
import ml_dtypes

BF16 = ml_dtypes.bfloat16
# 4 SWDGE queues (ucode max) — round-robin gathers across queue rings so
# their drains overlap; single_packet=True crashes HW, keep False.
NQ = 4
SP = False


def _rup(x, m):
    return (x + m - 1) // m * m


class CFG:
    def __init__(self, user=100000, item=50000, d=64, e=3200000, b=16384,
                 ncores=8, chunk=25088):
        self.USER, self.ITEM, self.D, self.E, self.B = user, item, d, e, b
        self.NC = ncores
        self.CHUNK = chunk
        self.UPAD = _rup(user, 128 * ncores)
        self.IPAD = _rup(item, 128 * ncores)
        self.UCH = (self.UPAD + chunk - 1) // chunk
        self.ICH = (self.IPAD + chunk - 1) // chunk
        self.USH = self.UPAD // ncores
        self.ISH = self.IPAD // ncores
        self.UBLK = self.USH // 128
        self.IBLK = self.ISH // 128
        self.BSH = b // ncores
        assert b % (16 * ncores) == 0


def _pieces(nblk):
    if nblk < 2:
        return [(0, nblk)]
    h = max(1, min(nblk - 1, (nblk * 3) // 4))
    return [(0, h), (h, nblk)]


def _src_segmap(cfg, src, sh_rows, pieces):
    """Map source row ids to (seg, sloc) over the piece-ordered table.

    Piece-ordered table: rows = concat over pieces of concat over cores of
    the core's piece-slice. Returns seg, sloc, specs [(piece, lo, sz)],
    bases [row base of each piece in a single-tensor layout].
    """
    core_s = src // sh_rows
    m = src % sh_rows
    seg = np.zeros(len(src), np.int64)
    sloc = np.zeros(len(src), np.int64)
    specs, bases = [], []
    seg_base = 0
    tbl_base = 0
    for pidx, (b0, b1) in enumerate(pieces):
        rlo, rhi = b0 * 128, b1 * 128
        prows = (rhi - rlo) * cfg.NC
        nch = (prows + 32767) // 32768 if prows > 32767 else 1
        nch = max(nch, (prows + 32767) // 32767)
        csz = _rup((prows + nch - 1) // nch, 16)
        msk = (m >= rlo) & (m < rhi)
        pos = core_s[msk] * (rhi - rlo) + (m[msk] - rlo)
        seg[msk] = seg_base + pos // csz
        sloc[msk] = pos % csz
        for c in range(nch):
            lo = c * csz
            specs.append((pidx, lo, min(csz, prows - lo)))
        bases.append(tbl_base)
        seg_base += nch
        tbl_base += prows
    return seg, sloc, specs, bases


def _prep_direction(cfg, dst, seg_in, sloc_in, val, sh_rows, nblk, nsch):
    """Token schedule for one direction (dst-sharded).

    Tokens sorted by (core, blk, sch, dloc); slot (p=rank%128,
    chunk=rank//128) within each (blk,sch) group, caps uniform across cores.
    Windows: per chunk, the half-blocks (0-63 / 64-127) its tokens touch in
    ANY core; one-hot = is_eq(dloc-64h, iota64)*val masks naturally.

    Returns (meta, per_core):
      meta: blocks -> dict(goffs=[(choff,cap)]*nsch, wins=[(cj,half)],
            gstart/gstop flags, woff, nch)
      per_core: idx16 [128,T/16] i16, wdst/wval [128,NW] bf16
    """
    NC = cfg.NC
    core = dst // sh_rows
    blk = (dst % sh_rows) // 128
    dloc = dst % 128
    sch = seg_in
    sloc = sloc_in.astype(np.int16)

    order = np.lexsort((dloc, sch, blk, core))
    core, blk, dloc, sch, sloc, val = (a[order] for a in
                                       (core, blk, dloc, sch, sloc, val))

    key = (core * nblk + blk) * nsch + sch
    ncell = NC * nblk * nsch
    counts = np.bincount(key, minlength=ncell).reshape(NC, nblk, nsch)
    caps = np.maximum(_rup(counts.max(axis=0), 128), 128)   # [nblk, nsch]

    # chunk offsets (in tokens) per (blk, sch)
    offs = np.zeros((nblk, nsch), np.int64)
    t = 0
    for b in range(nblk):
        for c in range(nsch):
            offs[b, c] = t
            t += caps[b, c]
    T = t
    NCHT = T // 128

    seg_start = np.zeros(ncell + 1, np.int64)
    np.cumsum(np.bincount(key, minlength=ncell), out=seg_start[1:])
    rank = np.arange(len(dst)) - seg_start[key]
    pos = offs[blk, sch] + rank
    cid = pos // 128          # global chunk id within core slab
    p = pos % 128

    idx_all = np.zeros((NC, T), np.int16)
    val_all = np.zeros((NC, T), np.float32)
    dloc_all = np.full((NC, T), -1, np.int16)
    idx_all[core, pos] = sloc
    val_all[core, pos] = val
    dloc_all[core, pos] = dloc

    # per-chunk half membership unioned over cores
    lo = np.full(NCHT, 127, np.int64)
    hi = np.zeros(NCHT, np.int64)
    np.minimum.at(lo, cid, dloc)
    np.maximum.at(hi, cid, dloc)
    has0 = lo < 64
    has1 = hi >= 64

    blocks = []
    nw = 0
    for b in range(nblk):
        c0 = offs[b, 0] // 128
        cend = (offs[b, 0] + caps[b].sum()) // 128
        wins, gstart, gstop = [], [], []
        for h in range(2):
            hs = [cj - c0 for cj in range(c0, cend)
                  if (has0[cj] if h == 0 else has1[cj])]
            if not hs:
                hs = [0]          # dummy: all-masked window still clears psum
            for i, cj in enumerate(hs):
                wins.append((cj, h))
                gstart.append(i == 0)
                gstop.append(i == len(hs) - 1)
        mins = counts.min(axis=0)
        blocks.append(dict(
            c0=c0, nch=cend - c0, woff=nw, wins=wins,
            gstart=gstart, gstop=gstop,
            goffs=[(int(offs[b, c]), int(caps[b, c])) for c in range(nsch)],
            glos=[int(mins[b, c]) // 128 for c in range(nsch)],
        ))
        nw += len(wins)

    # window descriptors (same for all cores): absolute chunk + half
    cj_abs = np.array([bm["c0"] + cj for bm in blocks
                       for (cj, h) in bm["wins"]], np.int64)
    hh = np.array([h for bm in blocks for (cj, h) in bm["wins"]], np.int64)

    per_core = []
    for k in range(NC):
        dw = dloc_all[k].reshape(NCHT, 128).T  # [128, NCHT] int16
        vw = val_all[k].reshape(NCHT, 128).T   # [128, NCHT] f32
        d = dw[:, cj_abs].astype(np.int64) - 64 * hh[None, :]  # [128, nw]
        v = vw[:, cj_abs]
        m = (d >= 0) & (d < 64)
        wone = np.zeros((128, nw, 64), BF16)
        pp, ww = np.nonzero(m)
        wone[pp, ww, d[pp, ww]] = v[pp, ww].astype(BF16)
        iw = idx_all[k].reshape(T // 16, 16).T
        idx16 = np.tile(iw, (8, 1))
        per_core.append(dict(idx=idx16, wone=wone.reshape(128, nw * 64)))

    meta = dict(T=T, NW=nw, blocks=blocks, nsch=nsch, nblk=nblk)
    return meta, per_core


def _wrap_shard(tbl_pad, k, sh_rows):
    s = tbl_pad[k * sh_rows:(k + 1) * sh_rows]
    nb = sh_rows // 128
    d = s.shape[1]
    return s.reshape(nb, 128, d).transpose(1, 0, 2).reshape(128, nb * d).copy()


def _wrap_vec(vec_pad, k, sh_rows):
    s = vec_pad[k * sh_rows:(k + 1) * sh_rows]
    nb = sh_rows // 128
    return s.reshape(nb, 128).T.copy()


def _wrap_idx(ix, n):
    w = ix.astype(np.int16).reshape(n // 16, 16).T
    return np.tile(w, (8, 1))


def build_program(cfg, mu, mi, with_tail=True):
    import concourse.bass as bass
    import concourse.bacc as bacc
    import concourse.tile as tile
    from concourse import mybir

    D, NC = cfg.D, cfg.NC
    f32, bf16, i16 = mybir.dt.float32, mybir.dt.bfloat16, mybir.dt.int16
    AOT = mybir.AluOpType

    nc = bacc.Bacc("TRN2", target_bir_lowering=False, debug=False,
                   num_devices=NC, num_swdge_queues=NQ,
                   dynamic_dma_scratch_size=24576)

    # ---- I/O ----
    i0p = nc.dram_tensor("i0p", [cfg.IPAD, 128], bf16, kind="ExternalInput")
    u0h = nc.dram_tensor("u0h", [cfg.USH, 128], bf16,
                         kind="ExternalInput")   # u0 half prefilled
    uemb_sh = nc.dram_tensor("uemb_sh", [128, cfg.UBLK * D], f32, kind="ExternalInput")
    iemb_sh = nc.dram_tensor("iemb_sh", [128, cfg.IBLK * D], f32, kind="ExternalInput")
    oldu_sh = nc.dram_tensor("oldu_sh", [128, cfg.UBLK * D], f32, kind="ExternalInput")
    oldi_sh = nc.dram_tensor("oldi_sh", [128, cfg.IBLK * D], f32, kind="ExternalInput")
    nu_sh = nc.dram_tensor("nu_sh", [128, cfg.UBLK], f32, kind="ExternalInput")
    ni_sh = nc.dram_tensor("ni_sh", [128, cfg.IBLK], f32, kind="ExternalInput")
    idx_u = nc.dram_tensor("idx_u", [128, mu["T"] // 16], i16, kind="ExternalInput")
    idx_i = nc.dram_tensor("idx_i", [128, mi["T"] // 16], i16, kind="ExternalInput")
    wone_u = nc.dram_tensor("wone_u", [128, mu["NW"] * 64], bf16,
                            kind="ExternalInput")
    wone_i = nc.dram_tensor("wone_i", [128, mi["NW"] * 64], bf16,
                            kind="ExternalInput")
    BSH = cfg.BSH
    bidx_u = nc.dram_tensor("bidx_u", [128, BSH // 16], i16, kind="ExternalInput")
    bidx_i = nc.dram_tensor("bidx_i", [128, BSH // 16], i16, kind="ExternalInput")
    bidx_j = nc.dram_tensor("bidx_j", [128, BSH // 16], i16, kind="ExternalInput")
    bmask = nc.dram_tensor("bmask", [128, (BSH // 128) * 8], bf16, kind="ExternalInput")
    ones_in = nc.dram_tensor("ones", [128, 1], f32, kind="ExternalInput")
    out_d = nc.dram_tensor("out", [4], f32, kind="ExternalOutput")

    # ---- internal DRAM ----
    def internal(name, rows, cols, dt=bf16, shared=False):
        kw = {"addr_space": "Shared"} if shared else {}
        return nc.dram_tensor(name, [rows, cols], dt, kind="Internal", **kw)

    pcs_u, pcs_i = mu["dst_pieces"], mi["dst_pieces"]

    def piece_pair(name, pieces):
        agins, fulls = [], []
        for pidx, (b0, b1) in enumerate(pieces):
            sh = (b1 - b0) * 128
            agins.append(internal(f"agin_{name}{pidx}", sh, 128))
            fulls.append(internal(f"{name}_full{pidx}", sh * NC, 128,
                                  shared=True))
        return agins, fulls

    agin_ug1, ug1_full = piece_pair("ug1", pcs_u)
    agin_gg, gg_full = piece_pair("gg", pcs_i)
    agin_g2u, g2u_full = piece_pair("g2u", pcs_u)
    agin_gcu = internal("agin_gcu", cfg.USH, D)
    gcu_full = internal("gcu_full", cfg.UPAD, D, shared=True)
    agin_gci = internal("agin_gci", cfg.ISH, D)
    gci_full = internal("gci_full", cfg.IPAD, D, shared=True)

    maxch = max(max(b["nch"] for b in mu["blocks"]),
                max(b["nch"] for b in mi["blocks"]))
    maxw = max(max(len(b["wins"]) for b in mu["blocks"]),
               max(len(b["wins"]) for b in mi["blocks"]))

    with tile.TileContext(nc) as tc:
        with (
            tc.tile_pool(name="persist", bufs=1) as pp,
            tc.tile_pool(name="io", bufs=3) as iop,
            tc.tile_pool(name="gath", bufs=3) as gp,
            tc.tile_pool(name="lhs", bufs=3) as lp,
            tc.tile_pool(name="drain", bufs=4) as dp,
            tc.tile_pool(name="psum", bufs=3, space="PSUM") as psp,
            tc.tile_pool(name="psumB", bufs=3, space="PSUM") as pspB,
            tc.tile_pool(name="psum4", bufs=1, space="PSUM") as psp4,
            tc.tile_pool(name="tail", bufs=1) as tp,
        ):
            gcn_u = pp.tile([128, cfg.UBLK, D], f32, tag="gcn_u")
            gcn_i = pp.tile([128, cfg.IBLK, D], f32, tag="gcn_i")
            qctr = [0]

            for pidx, (b0, b1) in enumerate(pcs_u):
                nc.sync.dma_start(agin_ug1[pidx].ap(),
                                  u0h.ap()[b0 * 128:b1 * 128, :])
            nc.sync.dma_start(gcn_u[:], uemb_sh.ap().rearrange(
                "p (b d) -> p b d", d=D))
            nc.sync.dma_start(gcn_i[:], iemb_sh.ap().rearrange(
                "p (b d) -> p b d", d=D))

            def gat(g_t, srcs, idx_t, bm, b, nsch):
                """Emit gathers for one block: per src segment, split into
                sub-gathers that fit one SWDGE queue ring (1024 descs)."""
                SPLIT = 4096
                off0 = bm["goffs"][0][0]
                for c, (off, cap) in enumerate(bm["goffs"]):
                    if cap == 0:
                        continue
                    rel = off - off0
                    s_t, s_lo, s_hi = srcs[c]
                    for o in range(0, cap, SPLIT):
                        sc = min(SPLIT, cap - o)
                        ro = rel + o
                        q = qctr[0] % NQ
                        qctr[0] += 1
                        nc.gpsimd.dma_gather(
                            g_t[:, ro // 128:(ro + sc) // 128, :],
                            s_t.ap()[s_lo:s_hi, :],
                            idx_t[:, ro // 16:(ro + sc) // 16],
                            num_idxs=sc,
                            num_idxs_reg=sc,
                            elem_size=128,
                            single_packet=SP,
                            queue_num=q,
                        )

            def hop_pass(meta, idx_d, wone_d, src_tensors, fused,
                         wgtA, accA, wgtB=None, accB=None,
                         tblA=None, tblA_col=None, tblB=None, tblB_col=None,
                         ag_after=None):
                """One gather pass. fused: two hops from row halves.

                src_tensors: one tensor per source piece (or a single tensor
                covering all pieces); tblA/tblB: list of per-piece agin
                tensors aligned with this direction's dst pieces; ag_after:
                {last_block_of_piece: [(agin, agout)]} collectives to emit.
                """
                specs = meta["specs"]
                if not isinstance(src_tensors, (list, tuple)):
                    bases = []
                    base = 0
                    for pidx, (b0, b1) in enumerate(meta["src_pieces"]):
                        bases.append(base)
                        base += (b1 - b0) * 128 * NC
                    srcs = [(src_tensors, bases[p] + lo, bases[p] + lo + sz)
                            for (p, lo, sz) in specs]
                else:
                    srcs = [(src_tensors[p], lo, lo + sz)
                            for (p, lo, sz) in specs]
                dpieces = meta["dst_pieces"]

                def tbl_at(tbl_list, b):
                    for (b0, b1), t in zip(dpieces, tbl_list):
                        if b0 <= b < b1:
                            return t, (b - b0) * 128
                    raise AssertionError

                for b, bm in enumerate(meta["blocks"]):
                    Tb = sum(c for _, c in bm["goffs"])
                    nwb = len(bm["wins"])
                    off0 = bm["goffs"][0][0]
                    idx_t = iop.tile([128, (maxch * 128) // 16], i16, tag="idx")
                    nc.sync.dma_start(
                        idx_t[:, :Tb // 16],
                        idx_d.ap()[:, off0 // 16:(off0 + Tb) // 16])

                    g_t = gp.tile([128, maxch, 128], bf16, tag="g")
                    gat(g_t, srcs, idx_t, bm, b, meta["nsch"])

                    l_t = lp.tile([128, maxw, 64], bf16, tag="l")
                    nc.sync.dma_start(
                        l_t[:, :nwb, :],
                        wone_d.ap()[:, bm["woff"] * 64:(bm["woff"] + nwb) * 64]
                        .rearrange("p (w j) -> p w j", j=64))

                    psA = psp.tile([128, D], f32, tag="psA")
                    if fused:
                        psB = pspB.tile([128, D], f32, tag="psB")
                    else:
                        psB = None
                    for w, (cj, h) in enumerate(bm["wins"]):
                        st, sp = bm["gstart"][w], bm["gstop"][w]
                        nc.tensor.matmul(
                            psA[64 * h:64 * h + 64, :],
                            l_t[:, w, :],
                            g_t[:, cj, 0:64],
                            start=st, stop=sp,
                            tile_position=(0, 64 * h),
                        )
                        if fused:
                            nc.tensor.matmul(
                                psB[64 * h:64 * h + 64, :],
                                l_t[:, w, :],
                                g_t[:, cj, 64:128],
                                start=st, stop=sp,
                                tile_position=(0, 64 * h),
                            )

                    def drain(ps, wgt, acc, tbl, tbl_col):
                        nc.vector.scalar_tensor_tensor(
                            acc[:, b, :], ps[:], float(wgt),
                            acc[:, b, :], AOT.mult, AOT.add)
                        if tbl is not None:
                            t, r0 = tbl_at(tbl, b)
                            cv = dp.tile([128, D], bf16, tag="cv")
                            nc.scalar.copy(cv[:], ps[:])
                            nc.sync.dma_start(
                                t.ap()[r0:r0 + 128, tbl_col:tbl_col + D],
                                cv[:])
                    drain(psA, wgtA, accA, tblA, tblA_col)
                    if fused:
                        drain(psB, wgtB, accB, tblB, tblB_col)
                    if ag_after and b in ag_after:
                        for gi, go in ag_after[b]:
                            allgather(gi, go)

            def allgather(ag_in, ag_out):
                nc.gpsimd.collective_compute(
                    "AllGather", mybir.AluOpType.bypass,
                    replica_groups=[list(range(NC))],
                    ins=[ag_in.ap()], outs=[ag_out.ap()],
                )

            U = (mu, idx_u, wone_u)
            I = (mi, idx_i, wone_i)

            def ag_map(pieces, agins, fulls):
                return {b1 - 1: [(agins[p], fulls[p])]
                        for p, (b0, b1) in enumerate(pieces)}

            def self_loss(acc_tile, old_d, n_d, nblk, col):
                CH = min(7, nblk)
                nv_t = tp.tile([128, nblk], f32, tag=f"nv{col}")
                rs = tp.tile([128, nblk], f32, tag=f"rs{col}")
                nc.sync.dma_start(nv_t[:], n_d.ap())
                for o in range(0, nblk, CH):
                    ch = min(CH, nblk - o)
                    old_t = iop.tile([128, CH, D], f32, tag="oldc")
                    nc.sync.dma_start(
                        old_t[:, :ch, :],
                        old_d.ap()[:, o * D:(o + ch) * D]
                        .rearrange("p (b d) -> p b d", d=D))
                    nc.vector.tensor_tensor(old_t[:, :ch, :],
                                            acc_tile[:, o:o + ch, :],
                                            old_t[:, :ch, :], AOT.subtract)
                    nc.vector.tensor_tensor(old_t[:, :ch, :],
                                            old_t[:, :ch, :],
                                            old_t[:, :ch, :], AOT.mult)
                    nc.vector.tensor_reduce(rs[:, o:o + ch], old_t[:, :ch, :],
                                            mybir.AxisListType.X, AOT.add)
                nc.scalar.activation(rs[:], rs[:],
                                     mybir.ActivationFunctionType.Sqrt)
                nc.vector.tensor_tensor(rs[:], rs[:], nv_t[:], AOT.mult)
                nc.vector.tensor_reduce(part_t[:, col:col + 1], rs[:],
                                        mybir.AxisListType.X, AOT.add)

            self_loss_fns = [
                lambda: self_loss(gcn_u, oldu_sh, nu_sh, cfg.UBLK, 2),
                lambda: self_loss(gcn_i, oldi_sh, ni_sh, cfg.IBLK, 3),
            ]

            # P1: g1u = A @ i0
            hop_pass(*U, i0p, False, 0.5, gcn_u,
                     tblA=agin_ug1, tblA_col=64,
                     ag_after=ag_map(pcs_u, agin_ug1, ug1_full))
            # P2: g1i = At@u0 (cols 0:64), g2i = At@g1u (cols 64:128)
            hop_pass(*I, ug1_full, True, 0.5, gcn_i, 1.0 / 3.0,
                     gcn_i, tblA=agin_gg, tblA_col=0,
                     tblB=agin_gg, tblB_col=64,
                     ag_after=ag_map(pcs_i, agin_gg, gg_full))
            # P3: g2u = A@g1i, g3u = A@g2i (g3u drain fills unused col half)
            hop_pass(*U, gg_full, True, 1.0 / 3.0, gcn_u, 0.25,
                     gcn_u, tblA=agin_g2u, tblA_col=0,
                     tblB=agin_g2u, tblB_col=64,
                     ag_after=ag_map(pcs_u, agin_g2u, g2u_full))
            # gcn_u complete -> AG for tail (overlaps P4)
            gcu_b16 = tp.tile([128, cfg.UBLK, D], bf16, tag="gcu16")
            nc.scalar.copy(gcu_b16[:], gcn_u[:])
            nc.sync.dma_start(
                agin_gcu.ap().rearrange("(b p) d -> p b d", p=128), gcu_b16[:])
            allgather(agin_gcu, gcu_full)
            part_t = tp.tile([128, 4], f32, tag="part")
            if not with_tail:
                nc.vector.memset(part_t[:], 0.0)
            if with_tail:
                # U-side self-distillation overlaps P4 (gcn_u is final)
                self_loss_fns[0]()
            # P4: g3i = At @ g2u
            hop_pass(*I, g2u_full, False, 0.25, gcn_i)
            gci_b16 = tp.tile([128, cfg.IBLK, D], bf16, tag="gci16")
            nc.scalar.copy(gci_b16[:], gcn_i[:])
            nc.sync.dma_start(
                agin_gci.ap().rearrange("(b p) d -> p b d", p=128), gci_b16[:])
            allgather(agin_gci, gci_full)

            # ---------------- tail ----------------

            if with_tail:
                self_loss_fns[1]()

                BS = BSH // 128
                mask_t = tp.tile([128, 8 * BS], bf16, tag="bmask")
                nc.sync.dma_start(mask_t[:], bmask.ap())

                def batch_rows(src_full, rows_full, group, bidx_d, mask_lo,
                               ngrp, tag, qn=0, boff=0):
                    gt_full = tp.tile([128, BS * 4 * D], bf16, tag="bgshare")
                    gt = gt_full[:, boff:boff + BS * group * D].rearrange(
                        "p (s gd) -> p s gd", gd=group * D)
                    bix_t = tp.tile([128, BSH // 16], i16, tag=f"bx{tag}")
                    nc.sync.dma_start(bix_t[:], bidx_d.ap())
                    src2 = src_full.ap().rearrange("(a g) d -> a (g d)",
                                                   g=group)
                    nc.gpsimd.dma_gather(
                        gt[:], src2, bix_t[:],
                        num_idxs=BSH, num_idxs_reg=BSH, elem_size=group * D,
                        single_packet=False, queue_num=qn)
                    rt = tp.tile([128, BS, D], f32, tag=f"br{tag}")
                    tmp = tp.tile([128, BS, D], f32, tag="btshare")
                    for q in range(ngrp):
                        m_b = mask_t[:, (mask_lo + q) * BS:
                                     (mask_lo + q + 1) * BS]\
                            .broadcast_to([128, BS, D])
                        dstt = rt if q == 0 else tmp
                        nc.vector.tensor_tensor(
                            dstt[:], gt[:, :, q * D:(q + 1) * D], m_b,
                            AOT.mult)
                        if q > 0:
                            nc.vector.tensor_tensor(rt[:], rt[:], tmp[:],
                                                    AOT.add)
                    return rt

                u_t = batch_rows(gcu_full, cfg.UPAD, 4, bidx_u, 0, 4, "u",
                                 qn=1)
                ii_t = batch_rows(gci_full, cfg.IPAD, 2, bidx_i, 4, 2, "i",
                                  qn=2)
                ij_t = batch_rows(gci_full, cfg.IPAD, 2, bidx_j, 6, 2, "j",
                                  qn=3, boff=BS * 2 * D)

                pr = tp.tile([128, BS, D], f32, tag="pr")
                pi = tp.tile([128, BS], f32, tag="pi")
                pj = tp.tile([128, BS], f32, tag="pj")
                nc.vector.tensor_tensor(pr[:], u_t[:], ii_t[:], AOT.mult)
                nc.vector.tensor_reduce(pi[:], pr[:], mybir.AxisListType.X,
                                        AOT.add)
                nc.vector.tensor_tensor(pr[:], u_t[:], ij_t[:], AOT.mult)
                nc.vector.tensor_reduce(pj[:], pr[:], mybir.AxisListType.X,
                                        AOT.add)
                nc.vector.tensor_tensor(pi[:], pi[:], pj[:], AOT.subtract)
                bt = tp.tile([128, BS], f32, tag="bt2")
                nc.scalar.activation(bt[:], pi[:],
                                     mybir.ActivationFunctionType.Sigmoid)
                nc.scalar.activation(bt[:], bt[:],
                                     mybir.ActivationFunctionType.Ln,
                                     accum_out=part_t[:, 0:1])

                rg = tp.tile([128, BS], f32, tag="rg")
                rgt = tp.tile([128, BS], f32, tag="rgt")
                nc.vector.tensor_tensor(pr[:], u_t[:], u_t[:], AOT.mult)
                nc.vector.tensor_reduce(rg[:], pr[:], mybir.AxisListType.X,
                                        AOT.add)
                nc.vector.tensor_tensor(pr[:], ii_t[:], ii_t[:], AOT.mult)
                nc.vector.tensor_reduce(rgt[:], pr[:], mybir.AxisListType.X,
                                        AOT.add)
                nc.vector.tensor_tensor(rg[:], rg[:], rgt[:], AOT.add)
                nc.vector.tensor_tensor(pr[:], ij_t[:], ij_t[:], AOT.mult)
                nc.vector.tensor_reduce(rgt[:], pr[:], mybir.AxisListType.X,
                                        AOT.add)
                nc.vector.tensor_tensor(rg[:], rg[:], rgt[:], AOT.add)
                nc.vector.tensor_reduce(part_t[:, 1:2], rg[:],
                                        mybir.AxisListType.X, AOT.add)

            ones_t = tp.tile([128, 1], f32, tag="ones")
            nc.sync.dma_start(ones_t[:], ones_in.ap())
            ps4 = psp4.tile([4, 1], f32, tag="ps4")
            nc.tensor.matmul(ps4[:], part_t[:], ones_t[:],
                             start=True, stop=True)
            out_t = tp.tile([4, 1], f32, tag="out4")
            nc.scalar.copy(out_t[:], ps4[:])
            nc.sync.dma_start(out_d.ap().rearrange("(a b) -> a b", b=1),
                              out_t[:])

    nc.compile()
    return nc


def _balance_perm(deg, n, nc_, sh):
    """Serpentine-assign ids (ranked by degree) to (core, slot) so per-core
    per-degree-band token counts are nearly equal. Returns newpos[orig_id]
    -> position in the padded block-cyclic layout."""
    order = np.argsort(-deg, kind="stable")
    i = np.arange(n)
    grp = i // nc_
    lane = i % nc_
    core = np.where((grp % 2).astype(bool), nc_ - 1 - lane, lane)
    newpos = np.empty(n, np.int64)
    newpos[order] = core * sh + grp
    return newpos


def _preprocess(cfg, inputs):
    user = np.asarray(inputs["user"]).astype(np.int64)
    item_i = np.asarray(inputs["item_i"]).astype(np.int64)
    item_j = np.asarray(inputs["item_j"]).astype(np.int64)
    edge_u = np.asarray(inputs["edge_u"]).astype(np.int64)
    edge_i = np.asarray(inputs["edge_i"]).astype(np.int64)
    edge_val = np.asarray(inputs["edge_val"]).astype(np.float32)
    user_emb = np.asarray(inputs["user_emb"]).astype(np.float32)
    item_emb = np.asarray(inputs["item_emb"]).astype(np.float32)
    old_U = np.asarray(inputs["old_U_emb"]).astype(np.float32)
    old_I = np.asarray(inputs["old_I_emb"]).astype(np.float32)
    n_U = np.asarray(inputs["n_U"]).astype(np.float32)
    n_I = np.asarray(inputs["n_I"]).astype(np.float32)

    D = cfg.D

    posu = _balance_perm(np.bincount(edge_u, minlength=cfg.USER),
                         cfg.USER, cfg.NC, cfg.USH)
    posi = _balance_perm(np.bincount(edge_i, minlength=cfg.ITEM),
                         cfg.ITEM, cfg.NC, cfg.ISH)
    edge_u = posu[edge_u]
    edge_i = posi[edge_i]
    user = posu[user]
    item_i = posi[item_i]
    item_j = posi[item_j]

    def perm_rows(a, n, pos):
        out = np.zeros((n,) + a.shape[1:], a.dtype)
        out[pos] = a
        return out

    uemb_p = perm_rows(user_emb, cfg.UPAD, posu)
    iemb_p = perm_rows(item_emb, cfg.IPAD, posi)
    oldu_p = perm_rows(old_U, cfg.UPAD, posu)
    oldi_p = perm_rows(old_I, cfg.IPAD, posi)
    nu_p = perm_rows(n_U, cfg.UPAD, posu)
    ni_p = perm_rows(n_I, cfg.IPAD, posi)

    # piece layouts of the two source tables (by dst-shard block halves)
    pcs_u = _pieces(cfg.UBLK)
    pcs_i = _pieces(cfg.IBLK)
    # U direction: dst=user, src=item (item-table pieces)
    seg_u, sloc_u, specs_i, bases_i = _src_segmap(cfg, edge_i, cfg.ISH, pcs_i)
    mu, pc_u = _prep_direction(cfg, edge_u, seg_u, sloc_u, edge_val,
                               cfg.USH, cfg.UBLK, len(specs_i))
    # I direction: dst=item, src=user (user-table pieces)
    seg_i, sloc_i, specs_u, bases_u = _src_segmap(cfg, edge_u, cfg.USH, pcs_u)
    mi, pc_i = _prep_direction(cfg, edge_i, seg_i, sloc_i, edge_val,
                               cfg.ISH, cfg.IBLK, len(specs_u))
    mu["specs"], mu["src_pieces"], mu["dst_pieces"] = specs_i, pcs_i, pcs_u
    mi["specs"], mi["src_pieces"], mi["dst_pieces"] = specs_u, pcs_u, pcs_i

    # i0 in item-piece order
    i0p = np.zeros((cfg.IPAD, 128), BF16)
    base = 0
    for (b0, b1) in pcs_i:
        rlo, rhi = b0 * 128, b1 * 128
        psz = rhi - rlo
        for k in range(cfg.NC):
            i0p[base + k * psz:base + (k + 1) * psz, :D] = \
                iemb_p[k * cfg.ISH + rlo:k * cfg.ISH + rhi].astype(BF16)
        base += cfg.NC * psz

    ones = np.ones((128, 1), np.float32)

    in_maps = []
    BSH, BS = cfg.BSH, cfg.BSH // 128
    for k in range(cfg.NC):
        u0h = np.zeros((cfg.USH, 128), BF16)
        u0h[:, :D] = uemb_p[k * cfg.USH:(k + 1) * cfg.USH].astype(BF16)
        bs = slice(k * BSH, (k + 1) * BSH)
        bu, bi, bj = user[bs], item_i[bs], item_j[bs]
        masks = np.zeros((128, 8 * BS), BF16)
        for q in range(4):
            m = (bu % 4 == q).astype(np.float32).reshape(BS, 128).T
            masks[:, q * BS:(q + 1) * BS] = m
        for q in range(2):
            m = (bi % 2 == q).astype(np.float32).reshape(BS, 128).T
            masks[:, (4 + q) * BS:(5 + q) * BS] = m
            m = (bj % 2 == q).astype(np.float32).reshape(BS, 128).T
            masks[:, (6 + q) * BS:(7 + q) * BS] = m
        in_maps.append({
            "i0p": i0p, "u0h": u0h,
            "uemb_sh": _wrap_shard(uemb_p, k, cfg.USH),
            "iemb_sh": _wrap_shard(iemb_p, k, cfg.ISH),
            "oldu_sh": _wrap_shard(oldu_p, k, cfg.USH),
            "oldi_sh": _wrap_shard(oldi_p, k, cfg.ISH),
            "nu_sh": _wrap_vec(nu_p, k, cfg.USH),
            "ni_sh": _wrap_vec(ni_p, k, cfg.ISH),
            "idx_u": pc_u[k]["idx"], "wone_u": pc_u[k]["wone"],
            "idx_i": pc_i[k]["idx"], "wone_i": pc_i[k]["wone"],
            "ones": ones,
            "bidx_u": _wrap_idx(bu // 4, BSH),
            "bidx_i": _wrap_idx(bi // 2, BSH),
            "bidx_j": _wrap_idx(bj // 2, BSH),
            "bmask": masks,
        })
    return mu, mi, in_maps


def run(cfg, inputs, trace=False, use_sim=False, **bkw):
    from concourse import bass_utils
    mu, mi, in_maps = _preprocess(cfg, inputs)
    nc = build_program(cfg, mu, mi, **bkw)
    if use_sim:
        from concourse.bass_interp import MultiCoreSim
        sim = MultiCoreSim(nc, num_cores=cfg.NC, trace=False)
        cores = [sim.cores[i] for i in sorted(sim.cores)]
        for k, core in enumerate(cores):
            for name, arr in in_maps[k].items():
                core.tensor(name)[:] = arr
        sim.simulate(check_with_hw=False)

        class R:
            results = [{"out": np.array(core.tensor("out"))}
                       for core in cores]
            exec_time_ns = None
        res = R()
    else:
        res = bass_utils.run_bass_kernel_spmd(
            nc, in_maps, core_ids=list(range(cfg.NC)), trace=trace)
    parts = np.stack([res.results[k]["out"] for k in range(cfg.NC)])
    tot = parts.sum(axis=0)
    loss_bpr = -tot[0] / cfg.B + 1e-4 * tot[1] / cfg.B
    loss_self = tot[2] / cfg.USER + tot[3] / cfg.ITEM
    out = np.array([loss_bpr, 100.0 * loss_self, 1.0, 1.0], np.float32)
    return out, res


def kernel(**inputs):
    cfg = CFG()
    out, _ = run(cfg, inputs)
    return out

